# revision 13
# baseline (speedup 1.0000x reference)
"""Trainium2 Bass kernel for the Dense_Adapter module.

Reference computation:
    h = x @ w1 + b1                          # fc1, (L, BT, Ca)
    <temporal depthwise conv + triangular pairwise-diff mixing over T>
    out = gelu(.) @ w2 + b2 + x              # fc2 + residual

The conv + masked pairwise-diff collapses algebraically to a per-channel
8-term weighted sum over the clip's T axis:
    result[p, ch] = sum_s cmix[s, ch] * hraw[l=p+1, b*T+s, ch] + rbias[ch]
    y[l, bt]      = hraw[l, bt] + result[b, l-1]   (l >= 1)
    y[0, bt]      = hraw[1, bt]                    (cls token, no mixing)
    g             = gelu(y + b1)
    out           = g @ w2 + b2 + x
with cmix / rbias precomputed on the host from conv_w / conv_b / b1.

Sharding: data-parallel over BT (128 -> 8 cores x 16). A 16-wide BT slice
holds 2 whole clips (T=8), so the temporal mixing is core-local.

Layout: everything on-device lives in the transposed domain with bt-major
token order (tok = bt*197 + l). The host hands each core
xT = x_slice.transpose(1,0,2).reshape(3152, 768).T as [768, 3152] and
receives outT [768, 3152] back. Consequences:
  - every DMA is contiguous (>=1.5KB runs), zero on-chip transposes
  - each clip is a contiguous 1576-column span, so the whole pipeline
    (fc1 -> mixing -> gelu -> fc2 -> store) runs per-clip and clip 1's
    compute overlaps clip 0's stores and the tail of the input stream
  - fc1:  hT[ca, tok]  = w1.T @ xT          (6 K-tiles PSUM-accumulated)
  - fc2:  outT[d, tok] = w2.T @ gT
  - fused PSUM drain: outT = (psum + b2_per_partition) + xT in one
    scalar_tensor_tensor (b2 is per-partition in the [d, tok] layout)

Matmuls run as float32r (TF32-like PE fast path, 4x fp32 throughput; the
bit layout is f32 so it's a pure AP bitcast). The residual + x and all
elementwise math stay full fp32; only the small adapter branch sees the
reduced mantissa.
"""

import numpy as np

import concourse.mybir as mybir
from concourse import bacc, bass_utils
from concourse.tile import TileContext

FP32 = mybir.dt.float32
AF = mybir.ActivationFunctionType
OP = mybir.AluOpType

L, BT, D, CA, T = 197, 128, 768, 192, 8
NCORES = 8
BTL = BT // NCORES          # 16 bt per core
NCLIP = BTL // T            # 2 clips per core
TOK = L * BTL               # 3152 tokens per core, tok = bt*197 + l
P = 128
KT = D // P                 # 6 K-tiles for fc1
MT = D // P                 # 6 M-tiles for fc2
PTOK = L - 1                # 196 patch tokens
CLIPW = T * L               # 1576 columns per clip
CW = CLIPW // 4             # 394-wide matmul chunks, 4 per clip

CA_TILES = [(0, 128), (128, 64)]

# packed-params column offsets (all blocks stacked along the free dim of a
# single [128, PCOLS] tile; sub-128-row blocks occupy their top rows)
OFF_W2 = KT * CA                 # 1152: w2 rows 0..127
OFF_B2 = OFF_W2 + 2 * D          # after w2_hi block
OFF_CM0 = OFF_B2 + MT
OFF_CM1 = OFF_CM0 + T
OFF_RB0 = OFF_CM1 + T
OFF_RB1 = OFF_RB0 + 1
OFF_B10 = OFF_RB1 + 1
OFF_B11 = OFF_B10 + 1
PCOLS = OFF_B11 + 1

TRACE = False
LAST_RESULTS = None
_cache = {}

# float32r: PE truncates the multiply mantissa (TF32-like) but runs 4x
# faster than full fp32. The numpy layout is identical f32, so tensors that
# feed matmuls are declared float32r natively (the BIR verifier requires
# producer dtype == f32r for fp32r matmul operands); slices consumed as
# plain f32 by DVE/ACT are bitcast back.
F32R = mybir.dt.float32r


def _f32(ap):
    return ap.bitcast(FP32)


def _build():
    nc = bacc.Bacc(
        "TRN2", target_bir_lowering=False, debug=False, num_devices=NCORES
    )
    xt_d = nc.dram_tensor("xt", [D, TOK], F32R, kind="ExternalInput").ap()
    pr_d = nc.dram_tensor("prm", [P, PCOLS], F32R, kind="ExternalInput").ap()
    out_d = nc.dram_tensor("outt", [D, TOK], FP32, kind="ExternalOutput").ap()
    # dest view for merged per-chunk stores: [p, mt, tok]
    out_v = out_d.rearrange("(m p) t -> p m t", p=P)

    with TileContext(nc) as tc:
        with (
            tc.tile_pool(name="pers", bufs=1) as pers,
            tc.tile_pool(name="ob", bufs=3) as ob_pool,
            tc.tile_pool(name="hps", bufs=2, space="PSUM") as h_pool,
            tc.tile_pool(name="f2", bufs=3, space="PSUM") as f2_pool,
        ):
            # ---- params: one packed tile, two DMAs (w1 first) ----
            prm = pers.tile([P, PCOLS], F32R, tag="prm")
            nc.sync.dma_start(out=prm[:, :OFF_W2], in_=pr_d[:, :OFF_W2])
            w1_sb = [prm[:, kt * CA:(kt + 1) * CA] for kt in range(KT)]

            # xT loads: quarters, clip-0 first, interleaved with params
            xt_sb = [
                pers.tile([P, TOK], F32R, tag=f"xt_{kt}", name=f"xtsb{kt}")
                for kt in range(KT)
            ]

            def load_chunk(b, ch):
                lo = b * CLIPW + ch * CW
                hi = lo + CW
                for kt in range(KT):
                    nc.sync.dma_start(
                        out=xt_sb[kt][:, lo:hi],
                        in_=xt_d[kt * P:(kt + 1) * P, lo:hi],
                    )

            load_chunk(0, 0)
            nc.sync.dma_start(out=prm[:, OFF_W2:], in_=pr_d[:, OFF_W2:])
            for ch in range(1, 4):
                load_chunk(0, ch)
            for ch in range(4):
                load_chunk(1, ch)
            w2_sb0 = prm[:, OFF_W2:OFF_W2 + D]
            w2_sb1 = prm[:CA - P, OFF_W2 + D:OFF_W2 + 2 * D]
            b2_sb = _f32(prm[:, OFF_B2:OFF_B2 + MT])
            cm_sb = [_f32(prm[:, OFF_CM0:OFF_CM0 + T]),
                     _f32(prm[:CA - P, OFF_CM1:OFF_CM1 + T])]
            rb_sb = [_f32(prm[:, OFF_RB0:OFF_RB0 + 1]),
                     _f32(prm[:CA - P, OFF_RB1:OFF_RB1 + 1])]
            b1_sb = [_f32(prm[:, OFF_B10:OFF_B10 + 1]),
                     _f32(prm[:CA - P, OFF_B11:OFF_B11 + 1])]

            # ---- persistent activations ----
            h_sb = [
                pers.tile([cn, TOK], FP32, tag=f"h_{ci}", name=f"hsb{ci}")
                for ci, (c0, cn) in enumerate(CA_TILES)
            ]
            g_sb = [
                pers.tile([cn, TOK], F32R, tag=f"g_{ci}", name=f"gsb{ci}")
                for ci, (c0, cn) in enumerate(CA_TILES)
            ]
            r_sb = [
                pers.tile([cn, NCLIP * PTOK], FP32, tag=f"r_{ci}", name=f"rsb{ci}")
                for ci, (c0, cn) in enumerate(CA_TILES)
            ]

            def fc1_clip(b):
                for ch in range(4):
                    s0 = b * CLIPW + ch * CW
                    h_ps = [
                        h_pool.tile([cn, CW], FP32, tag=f"hps_{ci}",
                                    name=f"hps{ci}")
                        for ci, (c0, cn) in enumerate(CA_TILES)
                    ]
                    for kt in range(KT):
                        for ci, (c0, cn) in enumerate(CA_TILES):
                            nc.tensor.matmul(
                                h_ps[ci],
                                lhsT=w1_sb[kt][:, c0:c0 + cn],
                                rhs=xt_sb[kt][:, s0:s0 + CW],
                                start=(kt == 0),
                                stop=(kt == KT - 1),
                            )
                    for ci in range(2):
                        nc.scalar.copy(
                            out=h_sb[ci][:, s0:s0 + CW], in_=h_ps[ci]
                        )

            def mix_clip(b):
                # mixing + ybar all on DVE (TensorScalarPtr is DVE-only;
                # GPSIMD rejects AP-scalar operands at codegen)
                for ci, (c0, cn) in enumerate(CA_TILES):
                    eng = nc.vector
                    h, g = h_sb[ci], g_sb[ci]
                    rsl = r_sb[ci][:, b * PTOK:(b + 1) * PTOK]
                    eng.tensor_scalar(
                        rsl, h[:, 0:PTOK], 0.0, rb_sb[ci], OP.mult, OP.add,
                    )
                    for si in range(T):
                        c1 = (b * T + si) * L + 1  # col of token l=1
                        eng.scalar_tensor_tensor(
                            out=rsl, in0=h[:, c1:c1 + PTOK],
                            scalar=cm_sb[ci][:, si:si + 1],
                            in1=rsl, op0=OP.mult, op1=OP.add,
                        )
                    # per 2-temporal-position piece (= one 394-col fc2
                    # chunk... CW=394=2*197): cls copy, ybar add, gelu.
                    # Splitting lets each fc2 chunk start as soon as its
                    # piece is gelu'd instead of waiting for the whole clip.
                    # ybar builds in h (fp32, dead afterwards) so values
                    # are only rounded to f32r once, at gelu's write to g
                    hR = h.rearrange("p (r l) -> p r l", l=L)
                    for ch in range(4):
                        r0 = b * T + 2 * ch
                        # cls columns (l=0): h[c] = h[c+1]
                        eng.tensor_copy(
                            out=hR[:, r0:r0 + 2, 0:1].squeeze(2),
                            in_=hR[:, r0:r0 + 2, 1:2].squeeze(2),
                        )
                        # ybar: h[:, s, 1:197] += rsl (step-0 broadcast)
                        eng.tensor_add(
                            out=hR[:, r0:r0 + 2, 1:L],
                            in0=hR[:, r0:r0 + 2, 1:L],
                            in1=rsl.unsqueeze(1).broadcast_to([cn, 2, PTOK]),
                        )
                        s0 = b * CLIPW + ch * CW
                        nc.scalar.activation(
                            out=g[:, s0:s0 + CW], in_=h[:, s0:s0 + CW],
                            func=AF.Gelu, bias=b1_sb[ci], scale=1.0,
                        )

            def fc2_clip(b):
                for ch in range(4):
                    s0 = b * CLIPW + ch * CW
                    ob = ob_pool.tile([P, MT * CW], FP32, tag="ob")
                    for mt in range(MT):
                        ps = f2_pool.tile([P, CW], FP32, tag="f2")
                        nc.tensor.matmul(
                            ps, lhsT=w2_sb0[:, mt * P:(mt + 1) * P],
                            rhs=g_sb[0][:, s0:s0 + CW],
                            start=True, stop=False,
                        )
                        nc.tensor.matmul(
                            ps, lhsT=w2_sb1[:, mt * P:(mt + 1) * P],
                            rhs=g_sb[1][:, s0:s0 + CW],
                            start=False, stop=True,
                        )
                        osl = ob[:, mt * CW:(mt + 1) * CW]
                        xsl = _f32(xt_sb[mt][:, s0:s0 + CW])
                        if mt < 4:
                            # outT = (psum + b2) + xT fused on DVE
                            nc.vector.scalar_tensor_tensor(
                                out=osl, in0=ps, scalar=b2_sb[:, mt:mt + 1],
                                in1=xsl, op0=OP.add, op1=OP.add,
                            )
                        else:
                            # ACT drains PSUM (+b2), GPSIMD adds the residual
                            nc.scalar.activation(
                                out=osl, in_=ps, func=AF.Identity,
                                bias=b2_sb[:, mt:mt + 1], scale=1.0,
                            )
                            nc.gpsimd.tensor_add(out=osl, in0=osl, in1=xsl)
                    nc.sync.dma_start(
                        out=out_v[:, :, s0:s0 + CW],
                        in_=ob.rearrange("p (m w) -> p m w", m=MT),
                    )

            # clip-serial emission; the Tile scheduler re-orders per
            # engine by simulated readiness
            fc1_clip(0)
            mix_clip(0)
            fc2_clip(0)
            fc1_clip(1)
            mix_clip(1)
            fc2_clip(1)

    nc.compile()
    return nc


def _host_params(w1, b1, conv_w, conv_b, w2, b2):
    s = np.arange(T, dtype=np.float64)
    a0 = np.where(s <= T - 2, s + 1, 0.0)
    a1 = s
    a2 = np.where(s >= 1, s - 1, 0.0)
    a3 = -(T - 1.0 - s)
    denom = T * (T - 1) / 2.0  # 28
    cmix = (
        np.outer(a0, conv_w[:, 0]) + np.outer(a1, conv_w[:, 1])
        + np.outer(a2, conv_w[:, 2]) + a3[:, None]
    ) / denom  # [T, CA]
    rbias = cmix.sum(0) * b1.astype(np.float64) + conv_b
    return (
        np.ascontiguousarray(cmix.T.astype(np.float32)),          # [CA, T]
        np.ascontiguousarray(rbias.astype(np.float32)[:, None]),  # [CA, 1]
    )


def kernel(x, w1, b1, conv_w, conv_b, w2, b2):
    global LAST_RESULTS
    x = np.asarray(x, dtype=np.float32)
    w1 = np.ascontiguousarray(np.asarray(w1, np.float32))
    b1 = np.asarray(b1, np.float32)
    conv_w = np.asarray(conv_w, np.float32)
    conv_b = np.asarray(conv_b, np.float32)
    w2 = np.ascontiguousarray(np.asarray(w2, np.float32))
    b2 = np.asarray(b2, np.float32)

    cmix, rbias = _host_params(w1, b1, conv_w, conv_b, w2, b2)
    prm = np.zeros((P, PCOLS), np.float32)
    for kt in range(KT):
        prm[:, kt * CA:(kt + 1) * CA] = w1[kt * P:(kt + 1) * P, :]
    prm[:, OFF_W2:OFF_W2 + D] = w2[:P, :]
    prm[:CA - P, OFF_W2 + D:OFF_W2 + 2 * D] = w2[P:, :]
    prm[:, OFF_B2:OFF_B2 + MT] = b2.reshape(MT, P).T
    prm[:, OFF_CM0:OFF_CM0 + T] = cmix[:P]
    prm[:CA - P, OFF_CM1:OFF_CM1 + T] = cmix[P:]
    prm[:, OFF_RB0] = rbias[:P, 0]
    prm[:CA - P, OFF_RB1] = rbias[P:, 0]
    prm[:, OFF_B10] = b1[:P]
    prm[:CA - P, OFF_B11] = b1[P:]

    if "nc" not in _cache:
        _cache["nc"] = _build()
    nc = _cache["nc"]

    in_maps = []
    for c in range(NCORES):
        # bt-major token order: [16, 197, 768] -> [3152, 768] -> T
        xs = x[:, c * BTL:(c + 1) * BTL, :].transpose(1, 0, 2).reshape(TOK, D)
        in_maps.append({"xt": np.ascontiguousarray(xs.T), "prm": prm})
    res = bass_utils.run_bass_kernel_spmd(
        nc, in_maps, core_ids=list(range(NCORES)), trace=TRACE,
    )
    LAST_RESULTS = res
    out = np.empty((L, BT, D), np.float32)
    for c, r in enumerate(res.results):
        out[:, c * BTL:(c + 1) * BTL, :] = (
            r["outt"].T.reshape(BTL, L, D).transpose(1, 0, 2)
        )
    return out
